# revision 11
# baseline (speedup 1.0000x reference)
"""Trainium2 Bass kernel for nn_DecoderLayer_15891378995467.

Fast-weight (linear-attention) decoder layer:
  qkv = h @ W_qkv.T ; q,k1,k2,v per head ; phi = L1-normalized elu+1
  two causal linear attentions mixed by pi ; output proj ; residual ; LayerNorm.

Sharding: data-parallel over batch (64 = 8 cores x 8 local batches).

Numerics: the attention branch contributes <1% of the output norm
(residual dominates), so the heavy GEMMs run in fp8e4m3 with DoubleRow
(2 fp8/cell, K=256 per pass):
  - QKV projection: hT fp8 x W_qkv fp8 (weights pre-scaled by S=16)
  - phi stored as u' = S*(elu(x)+1) in fp8 (S cancels exactly in the
    q/k L1 normalizations, applied in f32 via reciprocals)
  - scores/apply in fp8 (normal mode), mix via bf16 diag matmuls
  - O-projection: ly fp8 (scaled S2=16) x W_o fp8 (scaled S3=16), DR
Residual + LayerNorm in f32 (h shipped as bf16).
Measured end-to-end rel err ~1.7e-3 (gate 2e-2).

Per-core inputs (host prepares):
  hT8  [128,4,2,8,256] f8  : h[l,b,m] at [p,ch,k,b,l], m=ch*256+k*128+p (DR pairs)
  hN   [256,8,1024] bf16   : natural slice for the residual
  w18  [128,4,2,4096] f8   : S*W_qkv reordered [q|k1|k2|v] head-major, DR pairs
  w28  [128,4,2,1024] f8   : S3*W_o.T, DR pairs over head-pairs
  pc   [256,16] f32        : S2*SCALE*clip(pi).T | S2*SCALE*(1-clip(pi)).T
  mask0 [128,256] f32      : causal mask for s-tile0 (tri | ones)
  mask1 [128,128] f32      : causal mask for (s1,t1)
  ident [128,128] f32, identb [128,128] bf16, sel8 [128,24,24] f8
"""

import math
import sys

if "/opt/trn_rl_repo" not in sys.path:
    sys.path.insert(0, "/opt/trn_rl_repo")

import numpy as np
import ml_dtypes

import concourse.bass as bass
import concourse.mybir as mybir
import concourse.tile as tile
from concourse.vector_clock import ScopedClock, VectorClock
from concourse.bass_utils import run_bass_kernel_spmd

F32 = mybir.dt.float32
BF16 = mybir.dt.bfloat16
F8 = mybir.dt.float8e4
AX = mybir.AxisListType
ALU = mybir.AluOpType
ACTF = mybir.ActivationFunctionType
DR = mybir.MatmulPerfMode.DoubleRow

H, DH, DM = 8, 128, 1024
SLEN, BSZ = 256, 64
NCORES = 8
BLOC = BSZ // NCORES  # 8 local batches per core
LB = BLOC * SLEN  # 2048 token columns per core
SCALE = 1.0 / DH**0.5
LN_EPS = 1e-5
NQKV = 4 * DM  # 4096
S = 16.0  # W_qkv scale == u' scale (cancels in L1 norms)
S2 = 16.0  # ly scale
S3 = 16.0  # W_o scale
LNS = math.log(S)


class SplitDrainTileContext(tile.TileContext):
    """This walrus build only encodes one sem-wait per Drain; split the
    tail drain into a chain of single-wait drains."""

    def _drain_and_barrier(self, tick_clock, wait_clock):
        vc_full = tick_clock.global_clock
        n = len(vc_full)
        procs = [i for i in range(n) if vc_full[i] > 0]
        groups = [procs[i : i + 1] for i in range(len(procs))] or [[]]
        for grp in groups:
            part = VectorClock([0] * n)
            for p in grp:
                part.require_at_least(p, vc_full[p])
            d = self.nc.sync.drain()
            wait_clock.add_sem_waits(d.ins, ScopedClock({None: part}))
        self.nc.all_engine_barrier()
        assert self.sems is not None
        popped = self.nc._tile_sem_poison_stack.pop()
        assert popped is self._sem_poison
        self.nc.clear_and_free_semaphores(list(self.sems.allocated().values()))
        self.nc.all_engine_barrier()
        self._split_multiwaits()

    def _split_multiwaits(self):
        """Walrus here encodes at most one sem-wait per instruction; hoist
        extra waits onto same-engine NOPs inserted just before."""
        fn = self.nc.m.functions[0]
        for bb in fn.blocks:
            insts = list(bb.instructions)
            if not any(
                i.sync_info is not None and len(i.sync_info.on_wait) > 1
                for i in insts
            ):
                continue
            new_insts = []
            for inst in insts:
                si = inst.sync_info
                if si is not None and len(si.on_wait) > 1:
                    waits = list(si.on_wait)
                    eng = self.nc.engines[inst.engine]
                    for w in waits[:-1]:
                        bi = eng.nop()
                        nop = bi.ins
                        cur = self.nc.cur_bb.bb.instructions
                        assert cur and cur[-1] is nop
                        cur.pop()
                        nop.sync_info = mybir.SyncInfo(on_wait=[w], on_update=[])
                        new_insts.append(nop)
                    inst.sync_info = mybir.SyncInfo(
                        on_wait=[waits[-1]], on_update=list(si.on_update)
                    )
                new_insts.append(inst)
            try:
                bb.instructions[:] = new_insts
            except TypeError:
                bb.instructions = new_insts


def build_program(passes=1):
    nc = bass.Bass("TRN2", target_bir_lowering=False, debug=False, num_devices=NCORES)

    hT8 = nc.declare_dram_parameter("hT8", [128, 4, 2, BLOC, SLEN], F8, isOutput=False)
    hN = nc.declare_dram_parameter("hN", [SLEN, BLOC, DM], BF16, isOutput=False)
    w18 = nc.declare_dram_parameter("w18", [128, 4, 2, NQKV], F8, isOutput=False)
    w28 = nc.declare_dram_parameter("w28", [128, 4, 2, DM], F8, isOutput=False)
    pc = nc.declare_dram_parameter("pc", [SLEN, 2 * H], F32, isOutput=False)
    mask0 = nc.declare_dram_parameter("mask0", [128, 256], F32, isOutput=False)
    mask1 = nc.declare_dram_parameter("mask1", [128, 128], F32, isOutput=False)
    ident = nc.declare_dram_parameter("ident", [128, 128], F32, isOutput=False)
    identb = nc.declare_dram_parameter("identb", [128, 128], BF16, isOutput=False)
    sel8 = nc.declare_dram_parameter("sel8", [128, 24, 24], F8, isOutput=False)
    out = nc.declare_dram_parameter("out", [SLEN, BLOC, DM], F32, isOutput=True)

    with SplitDrainTileContext(nc) as tc:
        for _ in range(passes):
            _emit(nc, tc, hT8, hN, w18, w28, pc, mask0, mask1, ident, identb,
                  sel8, out)
    return nc


def _emit(nc, tc, hT8, hN, w18, w28, pc, mask0, mask1, ident, identb, sel8, out):
    from contextlib import ExitStack

    ctx = ExitStack()
    with ctx:
        # ---- persistent pools (live across both phases) ----
        singles = ctx.enter_context(tc.tile_pool(name="singles", bufs=1))
        qk_pool = ctx.enter_context(tc.tile_pool(name="qk", bufs=1))
        v_pool = ctx.enter_context(tc.tile_pool(name="v", bufs=1))
        hN_pool = ctx.enter_context(tc.tile_pool(name="hN", bufs=1))
        st_pool = ctx.enter_context(tc.tile_pool(name="stats", bufs=1))
        sc_pool = ctx.enter_context(tc.tile_pool(name="sc", bufs=3))
        lon_pool = ctx.enter_context(tc.tile_pool(name="lon", bufs=6))
        dg_pool = ctx.enter_context(tc.tile_pool(name="dg", bufs=4))
        ly_pool = ctx.enter_context(tc.tile_pool(name="ly", bufs=2))
        x_pool = ctx.enter_context(tc.tile_pool(name="x", bufs=3))
        ln_pool = ctx.enter_context(tc.tile_pool(name="ln", bufs=4))

        # ---- constants ----
        m0_s = singles.tile([128, 256], F32)
        nc.sync.dma_start(out=m0_s[:], in_=mask0[:])
        m1_s = singles.tile([128, 128], F32)
        nc.sync.dma_start(out=m1_s[:], in_=mask1[:])
        id_s = singles.tile([128, 128], F32)
        nc.sync.dma_start(out=id_s[:], in_=ident[:])
        idb_s = singles.tile([128, 128], BF16)
        nc.sync.dma_start(out=idb_s[:], in_=identb[:])
        sel_s = singles.tile([128, 24, 24], F8)
        nc.sync.dma_start(out=sel_s[:], in_=sel8[:])
        pc_s = singles.tile([128, 2, 2 * H], F32)
        pc_v = pc.rearrange("(t p) n -> t p n", p=128)
        for t in range(2):
            nc.sync.dma_start(out=pc_s[:, t, :], in_=pc_v[t])
        eps_s = singles.tile([128, 1], F32)
        nc.vector.memset(eps_s[:], LN_EPS)
        lns_s = singles.tile([128, 1], F32)
        nc.vector.memset(lns_s[:], LNS)
        invS_s = singles.tile([128, 1], F32)
        nc.vector.memset(invS_s[:], 1.0 / S)
        w2_s = singles.tile([128, 4, 2, DM], F8)
        nc.sync.dma_start(out=w2_s[:], in_=w28[:])

        # ---- persistent data tiles ----
        qk_s = qk_pool.tile([128, 24, LB], F8)  # u' = S*(elu+1) transposed
        v_s = v_pool.tile([128, 2, BLOC, DM], F8)  # v natural [s | lt,b,h*dv]
        hN_s = hN_pool.tile([128, 2, BLOC, DM], BF16)
        hN_v = hN.rearrange("(t p) b m -> t p b m", p=128)
        for lt in range(2):
            nc.sync.dma_start(out=hN_s[:, lt, :, :], in_=hN_v[lt])
        rT_s = st_pool.tile([128, 16, 24], F32)  # 1/sums per (b*2+lt) tile
        c1_s = st_pool.tile([128, 16, H], F32)
        c2_s = st_pool.tile([128, 16, H], F32)

        # ================= FRONT: QKV + phi + sums =================
        fctx = ExitStack()
        with fctx:
            fsing = fctx.enter_context(tc.tile_pool(name="fsing", bufs=1))
            stg_pool = fctx.enter_context(tc.tile_pool(name="stg", bufs=3))
            ssb_pool = fctx.enter_context(tc.tile_pool(name="ssb", bufs=2))
            psF = fctx.enter_context(tc.tile_pool(name="psF", bufs=4, space="PSUM"))
            psS = fctx.enter_context(tc.tile_pool(name="psS", bufs=2, space="PSUM"))

            hT_s = fsing.tile([128, 4, 2, LB], F8)
            hT_v = hT8.rearrange("p c k b l -> p c k (b l)")
            nc.sync.dma_start(out=hT_s[:], in_=hT_v)
            w1_s = fsing.tile([128, 4, 2, NQKV], F8)
            nc.sync.dma_start(out=w1_s[:], in_=w18[:])

            # --- v natural: out[t_tile, vcols] via DR (stationary=hT pair) ---
            for tt in range(16):  # b*2+lt
                b, lt = tt // 2, tt % 2
                for vc in range(2):
                    pv = psF.tile([128, 512], F32, tag="pq")
                    for ch in range(4):
                        nc.tensor.matmul(
                            pv[:],
                            lhsT=hT_s[:, ch, :, tt * 128 : (tt + 1) * 128],
                            rhs=w1_s[:, ch, :, 3 * DM + vc * 512 : 3 * DM + (vc + 1) * 512],
                            start=(ch == 0),
                            stop=(ch == 3),
                            perf_mode=DR,
                        )
                    dst = v_s[:, lt, b, vc * 512 : (vc + 1) * 512]
                    if vc == 0:
                        nc.vector.tensor_scalar(
                            dst, pv[:], 1.0 / S, None, op0=ALU.mult
                        )
                    else:
                        nc.scalar.activation(
                            dst, pv[:], ACTF.Copy, scale=invS_s[:]
                        )

            # --- q/k1/k2 transposed: out[n_blk, t] via DR + phi ---
            for nb in range(24):
                for cc in range(4):
                    pq = psF.tile([128, 512], F32, tag="pq")
                    for ch in range(4):
                        nc.tensor.matmul(
                            pq[:],
                            lhsT=w1_s[:, ch, :, nb * 128 : (nb + 1) * 128],
                            rhs=hT_s[:, ch, :, cc * 512 : (cc + 1) * 512],
                            start=(ch == 0),
                            stop=(ch == 3),
                            perf_mode=DR,
                        )
                    # u' = relu(ps) + S*exp(min(ps,0)/S)  (= S*(elu(x)+1))
                    d_t = stg_pool.tile([128, 512], BF16, tag="d")
                    nc.vector.tensor_scalar(
                        d_t[:], pq[:], 0.0, 1.0 / S, op0=ALU.min, op1=ALU.mult
                    )
                    e_t = stg_pool.tile([128, 512], BF16, tag="e")
                    nc.scalar.activation(e_t[:], d_t[:], ACTF.Exp, bias=lns_s[:])
                    nc.vector.scalar_tensor_tensor(
                        qk_s[:, nb, cc * 512 : (cc + 1) * 512],
                        pq[:], 0.0, e_t[:], op0=ALU.max, op1=ALU.add,
                    )

            # --- sums of u' per (block j, token t) + reciprocals ---
            for c in range(4):
                pss = psS.tile([128, 512], F32, tag="pssum")
                for j in range(24):
                    nc.tensor.matmul(
                        pss[0:24, :],
                        lhsT=sel_s[:, j, :],
                        rhs=qk_s[:, j, c * 512 : (c + 1) * 512],
                        start=(j == 0),
                        stop=(j == 23),
                    )
                s_sb = ssb_pool.tile([128, 512], F32, tag="ssb")
                nc.vector.tensor_copy(s_sb[0:24, :], pss[0:24, :])
                for sub in range(4):
                    ttile = c * 4 + sub
                    ps_t = psS.tile([128, 24], F32, tag="pstp")
                    nc.tensor.transpose(
                        ps_t[:, 0:24],
                        s_sb[0:24, sub * 128 : (sub + 1) * 128],
                        id_s[0:24, 0:24],
                    )
                    nc.vector.reciprocal(rT_s[:, ttile, :], ps_t[:, 0:24])
                    nc.vector.tensor_tensor(
                        c1_s[:, ttile, :], pc_s[:, ttile % 2, 0:H],
                        rT_s[:, ttile, 0:H], op=ALU.mult,
                    )
                    nc.gpsimd.tensor_mul(
                        c2_s[:, ttile, :], pc_s[:, ttile % 2, H : 2 * H],
                        rT_s[:, ttile, 0:H],
                    )

        # ================= BACK: attention + O-proj + LN =================
        bctx = ExitStack()
        with bctx:
            # PSUM is bank-granular (8 x 2KB): pack logical tiles per bank.
            psB = bctx.enter_context(tc.tile_pool(name="psB", bufs=2, space="PSUM"))
            psL = bctx.enter_context(tc.tile_pool(name="psL", bufs=2, space="PSUM"))
            psM = bctx.enter_context(tc.tile_pool(name="psM", bufs=2, space="PSUM"))
            psO = bctx.enter_context(tc.tile_pool(name="psO", bufs=2, space="PSUM"))

            for b in range(BLOC):
                ly_t = ly_pool.tile([128, H, SLEN], F8, tag="ly")
                q0 = b * SLEN
                for hh in range(H):
                    lo_nat = []  # [i][lt] sbuf bf16 tiles [t,dv]
                    mxt = psM.tile([128, 256], F32, tag="mx")
                    for i in range(2):
                        jk = 8 + i * 8 + hh
                        # scoresT [s, t] (unnormalized, x S^2); ps0|ps1 share a bank
                        ps01 = psB.tile([128, 384], F32, tag="sc")
                        ps0 = ps01[:, 0:256]
                        ps1 = ps01[:, 256:384]
                        nc.tensor.matmul(
                            ps0,
                            lhsT=qk_s[:, jk, q0 : q0 + 128],
                            rhs=qk_s[:, hh, q0 : q0 + 256],
                            start=True, stop=True,
                        )
                        nc.tensor.matmul(
                            ps1,
                            lhsT=qk_s[:, jk, q0 + 128 : q0 + 256],
                            rhs=qk_s[:, hh, q0 + 128 : q0 + 256],
                            start=True, stop=True,
                        )
                        # normalize by 1/sum_k (per s) + causal mask -> fp8
                        sc0 = sc_pool.tile([128, 256], F8, tag="sc0")
                        nc.vector.scalar_tensor_tensor(
                            sc0[:], ps0, rT_s[:, b * 2, jk : jk + 1],
                            m0_s[:], op0=ALU.mult, op1=ALU.mult,
                        )
                        sc1 = sc_pool.tile([128, 128], F8, tag="sc1")
                        nc.vector.scalar_tensor_tensor(
                            sc1[:], ps1, rT_s[:, b * 2 + 1, jk : jk + 1],
                            m1_s[:], op0=ALU.mult, op1=ALU.mult,
                        )
                        # apply: lo[t, dv] = sc.T @ v; lp0|lp1 share a bank
                        blk = slice(hh * 128, (hh + 1) * 128)
                        lp = psL.tile([128, 256], F32, tag="lo")
                        lp0 = lp[:, 0:128]
                        lp1 = lp[:, 128:256]
                        nc.tensor.matmul(
                            lp0, lhsT=sc0[:, 0:128], rhs=v_s[:, 0, b, blk],
                            start=True, stop=True,
                        )
                        nc.tensor.matmul(
                            lp1, lhsT=sc0[:, 128:256], rhs=v_s[:, 0, b, blk],
                            start=True, stop=False,
                        )
                        nc.tensor.matmul(
                            lp1, lhsT=sc1[:], rhs=v_s[:, 1, b, blk],
                            start=False, stop=True,
                        )
                        l0 = lon_pool.tile([128, 128], BF16, tag="lon")
                        l1 = lon_pool.tile([128, 128], BF16, tag="lon")
                        if i == 0:
                            nc.scalar.copy(l0[:], lp0)
                            nc.vector.tensor_copy(l1[:], lp1)
                        else:
                            nc.vector.tensor_copy(l0[:], lp0)
                            nc.scalar.copy(l1[:], lp1)
                        lo_nat.append((l0, l1))
                    # mix via diag matmuls: ly[dv, t] = sum_i lo_i^T diag(c_i)
                    for lt in range(2):
                        mps = mxt[:, lt * 128 : (lt + 1) * 128]
                        for i in range(2):
                            c_s = c1_s if i == 0 else c2_s
                            dg = dg_pool.tile([128, 128], BF16, tag="dg")
                            nc.gpsimd.tensor_scalar(
                                dg[:], idb_s[:], c_s[:, b * 2 + lt, hh : hh + 1],
                                None, op0=ALU.mult,
                            )
                            nc.tensor.matmul(
                                mps,
                                lhsT=lo_nat[i][lt][:],
                                rhs=dg[:],
                                start=(i == 0),
                                stop=(i == 1),
                            )
                        nc.scalar.copy(
                            ly_t[:, hh, lt * 128 : (lt + 1) * 128], mps
                        )

                # O-proj (fp8 DR over head pairs) + residual + LN per l_tile
                for lt in range(2):
                    x_t = x_pool.tile([128, DM], F32, tag="x")
                    acc = ln_pool.tile([128, 4], F32, tag="acc")
                    for mo in range(2):
                        pso = psO.tile([128, 512], F32, tag="op")
                        for hp in range(4):
                            nc.tensor.matmul(
                                pso[:],
                                lhsT=ly_t[:, 2 * hp : 2 * hp + 2,
                                          lt * 128 : (lt + 1) * 128],
                                rhs=w2_s[:, hp, :, mo * 512 : (mo + 1) * 512],
                                start=(hp == 0),
                                stop=(hp == 3),
                                perf_mode=DR,
                            )
                        nc.vector.scalar_tensor_tensor(
                            x_t[:, mo * 512 : (mo + 1) * 512],
                            pso[:],
                            1.0 / (S2 * S3),
                            hN_s[:, lt, b, mo * 512 : (mo + 1) * 512],
                            op0=ALU.mult,
                            op1=ALU.add,
                            accum_out=acc[:, mo : mo + 1],
                        )
                    sq = ln_pool.tile([128, DM], BF16, tag="sq")
                    ssq = ln_pool.tile([128, 1], F32, tag="ssq")
                    nc.scalar.activation(
                        sq[:], x_t[:], ACTF.Square, accum_out=ssq[:]
                    )
                    mu = ln_pool.tile([128, 1], F32, tag="mu")
                    nc.vector.tensor_scalar(
                        mu[:], acc[:, 0:1], 1.0 / DM, None, op0=ALU.mult
                    )
                    nc.vector.scalar_tensor_tensor(
                        mu[:], acc[:, 1:2], 1.0 / DM, mu[:],
                        op0=ALU.mult, op1=ALU.add,
                    )
                    mu2 = ln_pool.tile([128, 1], F32, tag="mu2")
                    nc.vector.tensor_tensor(mu2[:], mu[:], mu[:], op=ALU.mult)
                    var = ln_pool.tile([128, 1], F32, tag="var")
                    nc.vector.scalar_tensor_tensor(
                        var[:], ssq[:], 1.0 / DM, mu2[:],
                        op0=ALU.mult, op1=ALU.subtract,
                    )
                    sd = ln_pool.tile([128, 1], F32, tag="sd")
                    nc.scalar.activation(sd[:], var[:], ACTF.Sqrt, bias=eps_s[:])
                    rstd = ln_pool.tile([128, 1], F32, tag="rstd")
                    nc.vector.reciprocal(rstd[:], sd[:])
                    nc.vector.tensor_scalar(
                        x_t[:], x_t[:], mu[:], rstd[:],
                        op0=ALU.subtract, op1=ALU.mult,
                    )
                    nc.sync.dma_start(
                        out=out[lt * 128 : (lt + 1) * 128, b, :], in_=x_t[:]
                    )


_PROGRAM_CACHE = {}


def _get_program():
    if "nc" not in _PROGRAM_CACHE:
        _PROGRAM_CACHE["nc"] = build_program()
    return _PROGRAM_CACHE["nc"]


def _to_bf16(x):
    """Fast f32 -> bf16 with round-to-nearest-even via uint tricks."""
    x = np.ascontiguousarray(x, dtype=np.float32)
    u = x.view(np.uint32)
    r = ((u >> 16) & 1).astype(np.uint32)
    out = ((u + 0x7FFF + r) >> 16).astype(np.uint16)
    return out.view(ml_dtypes.bfloat16)


def prepare_inputs(h, W_qkv, W_o, pi0, ln_gamma, ln_beta):
    """Host-side shard + relayout. Returns per-core input maps."""
    h = np.ascontiguousarray(h, dtype=np.float32)
    W_qkv = np.asarray(W_qkv, dtype=np.float32)
    W_o = np.asarray(W_o, dtype=np.float32)
    pi0 = np.asarray(pi0, dtype=np.float32)
    F8NP = ml_dtypes.float8_e4m3

    # w1: [m, g*1024 + h*128 + d] <- W_qkv[h*512? no: W_qkv[(h,g,d), m]]
    w1 = np.ascontiguousarray(
        W_qkv.reshape(H, 4, DH, DM).transpose(3, 1, 0, 2).reshape(DM, NQKV)
    )
    w18 = np.ascontiguousarray(
        (w1 * S).reshape(4, 2, 128, NQKV).transpose(2, 0, 1, 3)
    ).astype(F8NP)
    w2 = np.ascontiguousarray(W_o.T)  # [n = h*128+dv, m]
    w28 = np.ascontiguousarray(
        (w2 * S3).reshape(4, 2, 128, DM).transpose(2, 0, 1, 3)
    ).astype(F8NP)

    pi = np.clip(pi0[:, :SLEN], 0.0, 1.0)  # [H, SLEN]
    pcm = np.empty((SLEN, 2 * H), np.float32)
    pcm[:, :H] = (SCALE * S2) * pi.T
    pcm[:, H:] = (SCALE * S2) * (1.0 - pi.T)

    s_idx = np.arange(128)[:, None]
    l_idx = np.arange(256)[None, :]
    mask0 = (s_idx <= l_idx).astype(np.float32)  # [s0, t 0:256]
    mask1 = (s_idx <= l_idx[:, :128]).astype(np.float32)
    ident = np.eye(128, dtype=np.float32)
    identb = np.eye(128, dtype=ml_dtypes.bfloat16)
    selmat = np.zeros((128, 24, 24), dtype=F8NP)
    for j in range(24):
        selmat[:, j, j] = 1.0

    in_maps = []
    for c in range(NCORES):
        bsl = slice(c * BLOC, (c + 1) * BLOC)
        hc = h[:, bsl, :]  # [l, b, m]
        # hT8[p, ch, k, b, l] = h[l, b, ch*256+k*128+p]
        hT8 = np.ascontiguousarray(
            hc.transpose(2, 1, 0).reshape(4, 2, 128, BLOC, SLEN)
            .transpose(2, 0, 1, 3, 4)
        ).astype(F8NP)
        in_maps.append(
            dict(
                hT8=hT8,
                hN=_to_bf16(hc),
                w18=w18,
                w28=w28,
                pc=pcm,
                mask0=mask0,
                mask1=mask1,
                ident=ident,
                identb=identb,
                sel8=selmat,
            )
        )
    return in_maps


def finalize_output(results, ln_gamma, ln_beta):
    outs = [results[c]["out"] for c in range(NCORES)]
    full = np.concatenate(outs, axis=1)  # [SLEN, BSZ, DM]
    g = np.asarray(ln_gamma, dtype=np.float32)
    bta = np.asarray(ln_beta, dtype=np.float32)
    if not (np.all(g == 1.0) and np.all(bta == 0.0)):
        full = full * g + bta
    return full.astype(np.float32)


def kernel(h, W_qkv, W_o, pi0, ln_gamma, ln_beta):
    nc = _get_program()
    in_maps = prepare_inputs(h, W_qkv, W_o, pi0, ln_gamma, ln_beta)
    res = run_bass_kernel_spmd(nc, in_maps, list(range(NCORES)))
    return finalize_output(res.results, ln_gamma, ln_beta)


# revision 15
# speedup vs baseline: 2.0319x; 2.0319x over previous
"""Trainium2 Bass kernel for nn_DecoderLayer_15891378995467.

Fast-weight (linear-attention) decoder layer:
  qkv = h @ W_qkv.T ; q,k1,k2,v per head ; phi = L1-normalized elu+1
  two causal linear attentions mixed by pi ; output proj ; residual ; LayerNorm.

Sharding: data-parallel over batch (64 = 8 cores x 8 local batches).

Numerics: the attention branch contributes <1% of the output norm
(residual dominates), so the heavy GEMMs run in fp8e4m3 with DoubleRow
(2 fp8/cell, K=256 per pass):
  - QKV projection: hT fp8 x W_qkv fp8 (weights pre-scaled by S=16)
  - phi stored as u' = S*(elu(x)+1) in fp8 (S cancels exactly in the
    q/k L1 normalizations, applied in f32 via reciprocals)
  - scores/apply in fp8 (normal mode), mix via bf16 diag matmuls
  - O-projection: ly fp8 (scaled S2=16) x W_o fp8 (scaled S3=16), DR
Residual + LayerNorm in f32 (h shipped as bf16).
Measured end-to-end rel err ~1.7e-3 (gate 2e-2).

Per-core inputs (host prepares):
  hT8  [128,4,2,8,256] f8  : h[l,b,m] at [p,ch,k,b,l], m=ch*256+k*128+p (DR pairs)
  hN   [256,8,1024] bf16   : natural slice for the residual
  w18  [128,4,2,4096] f8   : S*W_qkv reordered [q|k1|k2|v] head-major, DR pairs
  w28  [128,4,2,1024] f8   : S3*W_o.T, DR pairs over head-pairs
  pc   [256,16] f32        : S2*SCALE*clip(pi).T | S2*SCALE*(1-clip(pi)).T
  mask0 [128,256] f32      : causal mask for s-tile0 (tri | ones)
  mask1 [128,128] f32      : causal mask for (s1,t1)
  ident [128,128] f32, identb [128,128] bf16, sel8 [128,24,24] f8
"""

import math
import sys

if "/opt/trn_rl_repo" not in sys.path:
    sys.path.insert(0, "/opt/trn_rl_repo")

import numpy as np
import ml_dtypes

import concourse.bass as bass
import concourse.mybir as mybir
import concourse.tile as tile
from concourse.vector_clock import ScopedClock, VectorClock
from concourse.bass_utils import run_bass_kernel_spmd

F32 = mybir.dt.float32
BF16 = mybir.dt.bfloat16
F8 = mybir.dt.float8e4
AX = mybir.AxisListType
ALU = mybir.AluOpType
ACTF = mybir.ActivationFunctionType
DR = mybir.MatmulPerfMode.DoubleRow

H, DH, DM = 8, 128, 1024
SLEN, BSZ = 256, 64
NCORES = 8
BLOC = BSZ // NCORES  # 8 local batches per core
LB = BLOC * SLEN  # 2048 token columns per core
SCALE = 1.0 / DH**0.5
LN_EPS = 1e-5
NQKV = 4 * DM  # 4096
S = 16.0  # W_qkv scale == u' scale (cancels in L1 norms)
S2 = 16.0  # ly scale
S3 = 16.0  # W_o scale
LNS = math.log(S)


class SplitDrainTileContext(tile.TileContext):
    """This walrus build only encodes one sem-wait per Drain; split the
    tail drain into a chain of single-wait drains."""

    def _drain_and_barrier(self, tick_clock, wait_clock):
        vc_full = tick_clock.global_clock
        n = len(vc_full)
        procs = [i for i in range(n) if vc_full[i] > 0]
        groups = [procs[i : i + 1] for i in range(len(procs))] or [[]]
        for grp in groups:
            part = VectorClock([0] * n)
            for p in grp:
                part.require_at_least(p, vc_full[p])
            d = self.nc.sync.drain()
            wait_clock.add_sem_waits(d.ins, ScopedClock({None: part}))
        self.nc.all_engine_barrier()
        assert self.sems is not None
        popped = self.nc._tile_sem_poison_stack.pop()
        assert popped is self._sem_poison
        self.nc.clear_and_free_semaphores(list(self.sems.allocated().values()))
        self.nc.all_engine_barrier()
        self._split_multiwaits()

    def _split_multiwaits(self):
        """Walrus here encodes at most one sem-wait per instruction; hoist
        extra waits onto same-engine NOPs inserted just before."""
        fn = self.nc.m.functions[0]
        for bb in fn.blocks:
            insts = list(bb.instructions)
            if not any(
                i.sync_info is not None and len(i.sync_info.on_wait) > 1
                for i in insts
            ):
                continue
            new_insts = []
            for inst in insts:
                si = inst.sync_info
                if si is not None and len(si.on_wait) > 1:
                    waits = list(si.on_wait)
                    eng = self.nc.engines[inst.engine]
                    for w in waits[:-1]:
                        bi = eng.nop()
                        nop = bi.ins
                        cur = self.nc.cur_bb.bb.instructions
                        assert cur and cur[-1] is nop
                        cur.pop()
                        nop.sync_info = mybir.SyncInfo(on_wait=[w], on_update=[])
                        new_insts.append(nop)
                    inst.sync_info = mybir.SyncInfo(
                        on_wait=[waits[-1]], on_update=list(si.on_update)
                    )
                new_insts.append(inst)
            try:
                bb.instructions[:] = new_insts
            except TypeError:
                bb.instructions = new_insts


def build_program(passes=1):
    nc = bass.Bass("TRN2", target_bir_lowering=False, debug=False, num_devices=NCORES)

    hT8 = nc.declare_dram_parameter("hT8", [128, 4, 2, BLOC, SLEN], F8, isOutput=False)
    hN = nc.declare_dram_parameter("hN", [SLEN, BLOC, DM], BF16, isOutput=False)
    w18 = nc.declare_dram_parameter("w18", [128, 4, 2, NQKV], F8, isOutput=False)
    w28 = nc.declare_dram_parameter("w28", [128, 4, 2, DM], F8, isOutput=False)
    pc = nc.declare_dram_parameter("pc", [SLEN, 2 * H], F32, isOutput=False)
    mask0 = nc.declare_dram_parameter("mask0", [128, 256], F32, isOutput=False)
    mask1 = nc.declare_dram_parameter("mask1", [128, 128], F32, isOutput=False)
    ident = nc.declare_dram_parameter("ident", [128, 128], F32, isOutput=False)
    identb = nc.declare_dram_parameter("identb", [128, 128], BF16, isOutput=False)
    sel8 = nc.declare_dram_parameter("sel8", [128, 24, 24], F8, isOutput=False)
    out = nc.declare_dram_parameter("out", [SLEN, BLOC, DM], F32, isOutput=True)

    with SplitDrainTileContext(nc) as tc:
        for _ in range(passes):
            _emit(nc, tc, hT8, hN, w18, w28, pc, mask0, mask1, ident, identb,
                  sel8, out)
    return nc


def _emit(nc, tc, hT8, hN, w18, w28, pc, mask0, mask1, ident, identb, sel8, out):
    from contextlib import ExitStack

    ctx = ExitStack()
    with ctx:
        # ---- persistent pools (live across both phases) ----
        singles = ctx.enter_context(tc.tile_pool(name="singles", bufs=1))
        qk_pool = ctx.enter_context(tc.tile_pool(name="qk", bufs=1))
        v_pool = ctx.enter_context(tc.tile_pool(name="v", bufs=1))
        hN_pool = ctx.enter_context(tc.tile_pool(name="hN", bufs=1))
        st_pool = ctx.enter_context(tc.tile_pool(name="stats", bufs=1))
        sc_pool = ctx.enter_context(tc.tile_pool(name="sc", bufs=3))
        lon_pool = ctx.enter_context(tc.tile_pool(name="lon", bufs=4))
        ly_pool = ctx.enter_context(tc.tile_pool(name="ly", bufs=2))
        x_pool = ctx.enter_context(tc.tile_pool(name="x", bufs=3))
        ln_pool = ctx.enter_context(tc.tile_pool(name="ln", bufs=4))

        # ---- constants ----
        m0_s = singles.tile([128, 256], F32)
        nc.sync.dma_start(out=m0_s[:], in_=mask0[:])
        m1_s = singles.tile([128, 128], F32)
        nc.sync.dma_start(out=m1_s[:], in_=mask1[:])
        id_s = singles.tile([128, 128], F32)
        nc.sync.dma_start(out=id_s[:], in_=ident[:])
        idb_s = singles.tile([128, 128], BF16)
        nc.sync.dma_start(out=idb_s[:], in_=identb[:])
        sel_s = singles.tile([128, 24, 24], F8)
        nc.sync.dma_start(out=sel_s[:], in_=sel8[:])
        pc_s = singles.tile([128, 2, 2 * H], F32)
        pc_v = pc.rearrange("(t p) n -> t p n", p=128)
        for t in range(2):
            nc.sync.dma_start(out=pc_s[:, t, :], in_=pc_v[t])
        eps_s = singles.tile([128, 1], F32)
        nc.vector.memset(eps_s[:], LN_EPS)
        lns_s = singles.tile([128, 1], F32)
        nc.vector.memset(lns_s[:], LNS)
        invS_s = singles.tile([128, 1], F32)
        nc.vector.memset(invS_s[:], 1.0 / S)
        ninvS_s = singles.tile([128, 1], F32)
        nc.vector.memset(ninvS_s[:], -1.0 / S)
        none_s = singles.tile([128, 1], F32)
        nc.vector.memset(none_s[:], -1.0)
        w2_s = singles.tile([128, 4, 2, DM], F8)
        nc.sync.dma_start(out=w2_s[:], in_=w28[:])

        # ---- persistent data tiles ----
        qk_s = qk_pool.tile([128, 24, LB], F8)  # u' = S*(elu+1) transposed
        v_s = v_pool.tile([128, 2, BLOC, DM], F8)  # v natural [s | lt,b,h*dv]
        hN_s = hN_pool.tile([128, 2, BLOC, DM], BF16)
        hN_v = hN.rearrange("(t p) b m -> t p b m", p=128)
        for lt in range(2):
            nc.sync.dma_start(out=hN_s[:, lt, :, :], in_=hN_v[lt])
        rT_s = st_pool.tile([128, 16, 24], F32)  # 1/sums per (b*2+lt) tile
        c1_s = st_pool.tile([128, 16, H], F32)
        c2_s = st_pool.tile([128, 16, H], F32)

        # ================= FRONT: QKV + phi + sums =================
        fctx = ExitStack()
        with fctx:
            fsing = fctx.enter_context(tc.tile_pool(name="fsing", bufs=1))
            stg_pool = fctx.enter_context(tc.tile_pool(name="stg", bufs=3))
            ssb_pool = fctx.enter_context(tc.tile_pool(name="ssb", bufs=2))
            psF = fctx.enter_context(tc.tile_pool(name="psF", bufs=4, space="PSUM"))
            psS = fctx.enter_context(tc.tile_pool(name="psS", bufs=2, space="PSUM"))

            hT_s = fsing.tile([128, 4, 2, LB], F8)
            hT_v = hT8.rearrange("p c k b l -> p c k (b l)")
            nc.sync.dma_start(out=hT_s[:], in_=hT_v)
            w1_s = fsing.tile([128, 4, 2, NQKV], F8)
            nc.sync.dma_start(out=w1_s[:], in_=w18[:])

            # --- v natural: out[t_tile, vcols] via DR (stationary=hT pair) ---
            for tt in range(16):  # b*2+lt
                b, lt = tt // 2, tt % 2
                for vc in range(2):
                    pv = psF.tile([128, 512], F32, tag="pq")
                    for ch in range(4):
                        nc.tensor.matmul(
                            pv[:],
                            lhsT=hT_s[:, ch, :, tt * 128 : (tt + 1) * 128],
                            rhs=w1_s[:, ch, :, 3 * DM + vc * 512 : 3 * DM + (vc + 1) * 512],
                            start=(ch == 0),
                            stop=(ch == 3),
                            perf_mode=DR,
                        )
                    dst = v_s[:, lt, b, vc * 512 : (vc + 1) * 512]
                    if vc == 0:
                        nc.vector.tensor_scalar(
                            dst, pv[:], 1.0 / S, None, op0=ALU.mult
                        )
                    else:
                        nc.scalar.activation(
                            dst, pv[:], ACTF.Copy, scale=invS_s[:]
                        )

            # --- q/k1/k2 transposed: out[n_blk, t] via DR + phi ---
            for nb in range(24):
                for cc in range(4):
                    pq = psF.tile([128, 512], F32, tag="pq")
                    for ch in range(4):
                        nc.tensor.matmul(
                            pq[:],
                            lhsT=w1_s[:, ch, :, nb * 128 : (nb + 1) * 128],
                            rhs=hT_s[:, ch, :, cc * 512 : (cc + 1) * 512],
                            start=(ch == 0),
                            stop=(ch == 3),
                            perf_mode=DR,
                        )
                    # u' = relu(ps) + S*exp(min(ps,0)/S)  (= S*(elu(x)+1))
                    # min(ps,0)/S = -relu(-ps/S): both phi activations on ACT
                    d_t = stg_pool.tile([128, 512], BF16, tag="d")
                    nc.scalar.activation(d_t[:], pq[:], ACTF.Relu, scale=ninvS_s[:])
                    e_t = stg_pool.tile([128, 512], BF16, tag="e")
                    nc.scalar.activation(
                        e_t[:], d_t[:], ACTF.Exp, scale=none_s[:], bias=lns_s[:]
                    )
                    nc.vector.scalar_tensor_tensor(
                        qk_s[:, nb, cc * 512 : (cc + 1) * 512],
                        pq[:], 0.0, e_t[:], op0=ALU.max, op1=ALU.add,
                    )

            # --- sums of u' per (block j, token t) + reciprocals ---
            for c in range(4):
                pss = psS.tile([128, 512], F32, tag="pssum")
                for j in range(24):
                    nc.tensor.matmul(
                        pss[0:24, :],
                        lhsT=sel_s[:, j, :],
                        rhs=qk_s[:, j, c * 512 : (c + 1) * 512],
                        start=(j == 0),
                        stop=(j == 23),
                    )
                s_sb = ssb_pool.tile([128, 512], F32, tag="ssb")
                nc.vector.tensor_copy(s_sb[0:24, :], pss[0:24, :])
                for sub in range(4):
                    ttile = c * 4 + sub
                    ps_t = psS.tile([128, 24], F32, tag="pstp")
                    nc.tensor.transpose(
                        ps_t[:, 0:24],
                        s_sb[0:24, sub * 128 : (sub + 1) * 128],
                        id_s[0:24, 0:24],
                    )
                    nc.vector.reciprocal(rT_s[:, ttile, :], ps_t[:, 0:24])
                    nc.vector.tensor_tensor(
                        c1_s[:, ttile, :], pc_s[:, ttile % 2, 0:H],
                        rT_s[:, ttile, 0:H], op=ALU.mult,
                    )
                    nc.gpsimd.tensor_mul(
                        c2_s[:, ttile, :], pc_s[:, ttile % 2, H : 2 * H],
                        rT_s[:, ttile, 0:H],
                    )

        # ================= BACK: attention + O-proj + LN =================
        bctx = ExitStack()
        with bctx:
            # PSUM is bank-granular (8 x 2KB): pack logical tiles per bank.
            psB = bctx.enter_context(tc.tile_pool(name="psB", bufs=2, space="PSUM"))
            psL = bctx.enter_context(tc.tile_pool(name="psL", bufs=2, space="PSUM"))
            psM = bctx.enter_context(tc.tile_pool(name="psM", bufs=2, space="PSUM"))
            psO = bctx.enter_context(tc.tile_pool(name="psO", bufs=2, space="PSUM"))

            for b in range(BLOC):
                ly_t = ly_pool.tile([128, H, SLEN], F8, tag="ly")
                q0 = b * SLEN
                for hh in range(H):
                    mxt = psM.tile([128, 256], F32, tag="mx")
                    # lsum[lt] accumulates c1*lo1 + c2*lo2 in SBUF (per-t scale
                    # is per-partition here, so it folds into the PSUM copies)
                    l0 = lon_pool.tile([128, 128], BF16, tag="lon")
                    l1 = lon_pool.tile([128, 128], BF16, tag="lon")
                    for i in range(2):
                        jk = 8 + i * 8 + hh
                        c_s = c1_s if i == 0 else c2_s
                        # scoresT [s, t] (unnormalized, x S^2); ps0|ps1 share a bank
                        ps01 = psB.tile([128, 384], F32, tag="sc")
                        ps0 = ps01[:, 0:256]
                        ps1 = ps01[:, 256:384]
                        nc.tensor.matmul(
                            ps0,
                            lhsT=qk_s[:, jk, q0 : q0 + 128],
                            rhs=qk_s[:, hh, q0 : q0 + 256],
                            start=True, stop=True,
                        )
                        nc.tensor.matmul(
                            ps1,
                            lhsT=qk_s[:, jk, q0 + 128 : q0 + 256],
                            rhs=qk_s[:, hh, q0 + 128 : q0 + 256],
                            start=True, stop=True,
                        )
                        # normalize by 1/sum_k (per s) + causal mask -> fp8
                        sc0 = sc_pool.tile([128, 256], F8, tag="sc0")
                        nc.vector.scalar_tensor_tensor(
                            sc0[:, 0:128], ps0[:, 0:128], rT_s[:, b * 2, jk : jk + 1],
                            m1_s[:], op0=ALU.mult, op1=ALU.mult,
                        )
                        nc.scalar.activation(
                            sc0[:, 128:256], ps0[:, 128:256], ACTF.Copy,
                            scale=rT_s[:, b * 2, jk : jk + 1],
                        )
                        sc1 = sc_pool.tile([128, 128], F8, tag="sc1")
                        nc.vector.scalar_tensor_tensor(
                            sc1[:], ps1, rT_s[:, b * 2 + 1, jk : jk + 1],
                            m1_s[:], op0=ALU.mult, op1=ALU.mult,
                        )
                        # apply: lo[t, dv] = sc.T @ v; lp0|lp1 share a bank
                        blk = slice(hh * 128, (hh + 1) * 128)
                        lp = psL.tile([128, 256], F32, tag="lo")
                        lp0 = lp[:, 0:128]
                        lp1 = lp[:, 128:256]
                        nc.tensor.matmul(
                            lp0, lhsT=sc0[:, 0:128], rhs=v_s[:, 0, b, blk],
                            start=True, stop=True,
                        )
                        nc.tensor.matmul(
                            lp1, lhsT=sc0[:, 128:256], rhs=v_s[:, 0, b, blk],
                            start=True, stop=False,
                        )
                        nc.tensor.matmul(
                            lp1, lhsT=sc1[:], rhs=v_s[:, 1, b, blk],
                            start=False, stop=True,
                        )
                        if i == 0:
                            nc.scalar.activation(
                                l0[:], lp0, ACTF.Copy,
                                scale=c_s[:, b * 2, hh : hh + 1],
                            )
                            nc.scalar.activation(
                                l1[:], lp1, ACTF.Copy,
                                scale=c_s[:, b * 2 + 1, hh : hh + 1],
                            )
                        else:
                            nc.vector.scalar_tensor_tensor(
                                l0[:], lp0, c_s[:, b * 2, hh : hh + 1],
                                l0[:], op0=ALU.mult, op1=ALU.add,
                            )
                            nc.vector.scalar_tensor_tensor(
                                l1[:], lp1, c_s[:, b * 2 + 1, hh : hh + 1],
                                l1[:], op0=ALU.mult, op1=ALU.add,
                            )
                    # transpose to ly[dv, t] via matmul against identity
                    for lt, ll in ((0, l0), (1, l1)):
                        mps = mxt[:, lt * 128 : (lt + 1) * 128]
                        nc.tensor.matmul(
                            mps, lhsT=ll[:], rhs=idb_s[:], start=True, stop=True,
                        )
                        nc.scalar.copy(
                            ly_t[:, hh, lt * 128 : (lt + 1) * 128], mps
                        )

                # O-proj (fp8 DR over head pairs) + residual + LN per l_tile
                for lt in range(2):
                    x_t = x_pool.tile([128, DM], F32, tag="x")
                    acc = ln_pool.tile([128, 4], F32, tag="acc")
                    for mo in range(2):
                        pso = psO.tile([128, 512], F32, tag="op")
                        for hp in range(4):
                            nc.tensor.matmul(
                                pso[:],
                                lhsT=ly_t[:, 2 * hp : 2 * hp + 2,
                                          lt * 128 : (lt + 1) * 128],
                                rhs=w2_s[:, hp, :, mo * 512 : (mo + 1) * 512],
                                start=(hp == 0),
                                stop=(hp == 3),
                                perf_mode=DR,
                            )
                        nc.vector.scalar_tensor_tensor(
                            x_t[:, mo * 512 : (mo + 1) * 512],
                            pso[:],
                            1.0 / (S2 * S3),
                            hN_s[:, lt, b, mo * 512 : (mo + 1) * 512],
                            op0=ALU.mult,
                            op1=ALU.add,
                            accum_out=acc[:, mo : mo + 1],
                        )
                    sq = ln_pool.tile([128, DM], BF16, tag="sq")
                    ssq = ln_pool.tile([128, 1], F32, tag="ssq")
                    nc.scalar.activation(
                        sq[:], x_t[:], ACTF.Square, accum_out=ssq[:]
                    )
                    mu = ln_pool.tile([128, 1], F32, tag="mu")
                    nc.vector.tensor_scalar(
                        mu[:], acc[:, 0:1], 1.0 / DM, None, op0=ALU.mult
                    )
                    nc.vector.scalar_tensor_tensor(
                        mu[:], acc[:, 1:2], 1.0 / DM, mu[:],
                        op0=ALU.mult, op1=ALU.add,
                    )
                    mu2 = ln_pool.tile([128, 1], F32, tag="mu2")
                    nc.vector.tensor_tensor(mu2[:], mu[:], mu[:], op=ALU.mult)
                    var = ln_pool.tile([128, 1], F32, tag="var")
                    nc.vector.scalar_tensor_tensor(
                        var[:], ssq[:], 1.0 / DM, mu2[:],
                        op0=ALU.mult, op1=ALU.subtract,
                    )
                    sd = ln_pool.tile([128, 1], F32, tag="sd")
                    nc.scalar.activation(sd[:], var[:], ACTF.Sqrt, bias=eps_s[:])
                    rstd = ln_pool.tile([128, 1], F32, tag="rstd")
                    nc.vector.reciprocal(rstd[:], sd[:])
                    nc.vector.tensor_scalar(
                        x_t[:], x_t[:], mu[:], rstd[:],
                        op0=ALU.subtract, op1=ALU.mult,
                    )
                    nc.sync.dma_start(
                        out=out[lt * 128 : (lt + 1) * 128, b, :], in_=x_t[:]
                    )


_PROGRAM_CACHE = {}


def _get_program():
    if "nc" not in _PROGRAM_CACHE:
        _PROGRAM_CACHE["nc"] = build_program()
    return _PROGRAM_CACHE["nc"]


def _to_bf16(x):
    """Fast f32 -> bf16 with round-to-nearest-even via uint tricks."""
    x = np.ascontiguousarray(x, dtype=np.float32)
    u = x.view(np.uint32)
    r = ((u >> 16) & 1).astype(np.uint32)
    out = ((u + 0x7FFF + r) >> 16).astype(np.uint16)
    return out.view(ml_dtypes.bfloat16)


def prepare_inputs(h, W_qkv, W_o, pi0, ln_gamma, ln_beta):
    """Host-side shard + relayout. Returns per-core input maps."""
    h = np.ascontiguousarray(h, dtype=np.float32)
    W_qkv = np.asarray(W_qkv, dtype=np.float32)
    W_o = np.asarray(W_o, dtype=np.float32)
    pi0 = np.asarray(pi0, dtype=np.float32)
    F8NP = ml_dtypes.float8_e4m3

    # w1: [m, g*1024 + h*128 + d] <- W_qkv[h*512? no: W_qkv[(h,g,d), m]]
    w1 = np.ascontiguousarray(
        W_qkv.reshape(H, 4, DH, DM).transpose(3, 1, 0, 2).reshape(DM, NQKV)
    )
    w18 = np.ascontiguousarray(
        (w1 * S).reshape(4, 2, 128, NQKV).transpose(2, 0, 1, 3)
    ).astype(F8NP)
    w2 = np.ascontiguousarray(W_o.T)  # [n = h*128+dv, m]
    w28 = np.ascontiguousarray(
        (w2 * S3).reshape(4, 2, 128, DM).transpose(2, 0, 1, 3)
    ).astype(F8NP)

    pi = np.clip(pi0[:, :SLEN], 0.0, 1.0)  # [H, SLEN]
    pcm = np.empty((SLEN, 2 * H), np.float32)
    pcm[:, :H] = (SCALE * S2) * pi.T
    pcm[:, H:] = (SCALE * S2) * (1.0 - pi.T)

    s_idx = np.arange(128)[:, None]
    l_idx = np.arange(256)[None, :]
    mask0 = (s_idx <= l_idx).astype(np.float32)  # [s0, t 0:256]
    mask1 = (s_idx <= l_idx[:, :128]).astype(np.float32)
    ident = np.eye(128, dtype=np.float32)
    identb = np.eye(128, dtype=ml_dtypes.bfloat16)
    selmat = np.zeros((128, 24, 24), dtype=F8NP)
    for j in range(24):
        selmat[:, j, j] = 1.0

    in_maps = []
    for c in range(NCORES):
        bsl = slice(c * BLOC, (c + 1) * BLOC)
        hc = h[:, bsl, :]  # [l, b, m]
        # hT8[p, ch, k, b, l] = h[l, b, ch*256+k*128+p]
        hT8 = np.ascontiguousarray(
            hc.transpose(2, 1, 0).reshape(4, 2, 128, BLOC, SLEN)
            .transpose(2, 0, 1, 3, 4)
        ).astype(F8NP)
        in_maps.append(
            dict(
                hT8=hT8,
                hN=_to_bf16(hc),
                w18=w18,
                w28=w28,
                pc=pcm,
                mask0=mask0,
                mask1=mask1,
                ident=ident,
                identb=identb,
                sel8=selmat,
            )
        )
    return in_maps


def finalize_output(results, ln_gamma, ln_beta):
    outs = [results[c]["out"] for c in range(NCORES)]
    full = np.concatenate(outs, axis=1)  # [SLEN, BSZ, DM]
    g = np.asarray(ln_gamma, dtype=np.float32)
    bta = np.asarray(ln_beta, dtype=np.float32)
    if not (np.all(g == 1.0) and np.all(bta == 0.0)):
        full = full * g + bta
    return full.astype(np.float32)


def kernel(h, W_qkv, W_o, pi0, ln_gamma, ln_beta):
    nc = _get_program()
    in_maps = prepare_inputs(h, W_qkv, W_o, pi0, ln_gamma, ln_beta)
    res = run_bass_kernel_spmd(nc, in_maps, list(range(NCORES)))
    return finalize_output(res.results, ln_gamma, ln_beta)
